# revision 24
# baseline (speedup 1.0000x reference)
"""Trainium2 Bass kernel for nn_DensityVQC (batched 2-qubit VQC Z-expectation).

Algebra
-------
The reference builds rho_b = conj(psi_b) psi_b^T (note: transpose of the
standard density matrix), evolves rho' = U rho U^dag and returns
tr(rho' Z0) with Z0 = diag(1,1,-1,-1).  This collapses to a per-row
quadratic form: with V = conj(U) (the transposed-rho convention flips the
conjugation) and phi = V psi,

    out_b = |phi_0|^2 + |phi_1|^2 - |phi_2|^2 - |phi_3|^2
          = 2 * || C psi_b ||^2 - ||psi_b||^2        (C = V[0:2, :], U unitary)
          = || A r_b + B m_b ||^2 - 1                (inputs are unit-norm)

with real 4x4 matrices A = sqrt(2)*[Re C; Im C], B = sqrt(2)*[-Im C; Re C].
So the device kernel is: per batch row (r, m in R^4), compute w = A r + B m,
then out = sum(w^2) - 1.  No [B,4,4] density matrices are ever materialized.

Device mapping (per core, pure data parallel over 8 cores)
----------------------------------------------------------
Everything on the data plane is fp16 (tolerance is 2e-2; fp16 input
quantization contributes ~1e-3).  This halves HBM traffic vs f32 (2.2 MB
in + 0.25 MB out per core) and runs the PE at full 1 col/cycle with FWL
weight loads.

Host-side marshalling reshapes each core's slice into component-major
layout [128 = 32 groups x 4 comps, 512] per supertile and packs the
input into per-chunk dram tensors, each a contiguous HBM block
(sequential reads - strided [128, wide] tensor rows measurably stall the
SDMA engines).  Chunks spread over THREE DMA rings (Sync + Scalar HWDGE
and GpSimd SWDGE): each ring stalls ~1us per chunk on write-receipts
before its semaphore fires, so overlapping rings keep the SDMA engines
fed.  Chunks are consumption-ordered; each chunk's semaphore gates
exactly the compute that needs it.

Per supertile (512 columns = 16384 batch rows), in arrival order
0,4,2,1,5,3,6,7:
  1. PE: phi = blkdiag32(A^T)^T . r + blkdiag32(B^T)^T . m   (two
     accumulating fp16 matmuls)
  2. ACT: S = phi^2 -> SBUF fp16
  3. PE: band reduce matmul: lhsT = blkdiag32(ones[4,1]) [128,32],
     tile_position=(0,32q) -> partitions [32q,32q+32) of the group's
     full-width PSUM bank (no 128-wide shifted patterns to DMA)
  4. DVE: copy with -1 fold -> fp16 out tile, DMA out (lo half issued
     mid-kernel; hi half is the tail)
A burst of N=128 dummy matmuls on a zeroed tile keeps the PE busy
through the DMA ramp so the HAM clock-gate releases (2.4 GHz) early.
The host un-permutes the [2,128,512] fp16 output back to batch order.
"""

import sys
import numpy as np

if "/opt/trn_rl_repo" not in sys.path:
    sys.path.insert(0, "/opt/trn_rl_repo")

import concourse.bass as bass
import concourse.tile as tile
from concourse import bacc, mybir
from concourse import bass_utils
from concourse.tile_rust import add_dep_helper

N_CORES = 8
BSZ = 1_048_576
BC = BSZ // N_CORES            # 131072 rows per core
NCOL = BC // 32                # 4096 component-major free columns
N_ST = NCOL // 512             # 8 supertiles
N_WARM = 34                    # N=128 PE warm-up matmuls during DMA ramp
                               # (~107 ns cadence; keeps the PE busy from the
                               # preamble barrier through the ~3.4 us HAM
                               # sustained-busy window so every real matmul
                               # runs at 2.4 GHz - overshooting s0's arrival
                               # costs ~0.1 us, undershooting costs ~2 us)
F32 = mybir.dt.float32
F16 = mybir.dt.float16
N_LAYERS = 6

# SBUF column layout of the packed input tile dd_t [128, 8480] (fp16).
COL_A = 0                      # ablk [*, 0:128]
COL_Z = 128                    # zsum [*, 128:160]
COL_B = 160                    # bblk [*, 160:288]
_ST_RCOL = {0: 288, 4: 1312, 2: 2336, 1: 3360, 5: 4384, 3: 5408, 6: 6432, 7: 7456}
ST_ORDER = [0, 4, 2, 1, 5, 3, 6, 7]
DD_COLS = 8480
# Chunks: (name, sbuf col range, ring).  Each is its own contiguous dram
# tensor.  Rings: 0 = Sync HWDGE, 1 = Scalar HWDGE, 2 = GpSimd SWDGE.
# The Scalar ring gets ONLY unthrottled early chunks: a throttled
# descgen in the Scalar FIFO would block the ACT table load and the
# whole square chain behind its semaphore wait.
CHUNKS = [
    ("s0", 0, 1312, 0),        # ablk | zsum | bblk | r0 | m0
    ("a0", 1312, 2336, 1),     # r4 | m4
    ("g0", 2336, 3360, 2),     # r2 | m2
    ("s1", 3360, 4384, 0),     # r1 | m1
    ("a1", 4384, 5408, 1),     # r5 | m5
    ("g1", 5408, 6432, 2),     # r3 | m3
    ("s2", 6432, 7456, 0),     # r6 | m6
    ("g2", 7456, 7968, 2),     # r7
    ("s3", 7968, 8480, 0),     # m7 (HWDGE ring: short sem->descgen->data lag)
]


def _circuit_unitary(ry, rz):
    """4x4 circuit unitary, float64 mirror of reference._circuit_unitary."""
    ry = np.asarray(ry, dtype=np.float64)
    rz = np.asarray(rz, dtype=np.float64)
    cnot = np.array(
        [[1, 0, 0, 0], [0, 1, 0, 0], [0, 0, 0, 1], [0, 0, 1, 0]],
        dtype=np.complex128,
    )

    def _ry(th):
        c, s = np.cos(th / 2), np.sin(th / 2)
        return np.array([[c, -s], [s, c]], dtype=np.complex128)

    def _rz(th):
        return np.diag([np.exp(-0.5j * th), np.exp(0.5j * th)])

    u = np.eye(4, dtype=np.complex128)
    for l in range(ry.shape[0]):
        ry_full = np.kron(_ry(ry[l, 0]), _ry(ry[l, 1]))
        rz_full = np.kron(_rz(rz[l, 0]), _rz(rz[l, 1]))
        u = cnot @ (rz_full @ (ry_full @ u))
    return u


def _host_consts(ry_params, rz_params):
    u = _circuit_unitary(ry_params, rz_params)
    c = np.conj(u)[0:2, :]
    a = np.sqrt(2.0) * np.vstack([c.real, c.imag])     # 4x4, w = A r + B m
    b = np.sqrt(2.0) * np.vstack([-c.imag, c.real])
    eye32 = np.eye(32, dtype=np.float32)
    # lhsT[k=4g+c, m=4g+j] = A[j, c]  ->  block_diag of A.T
    ablk = np.kron(eye32, a.T.astype(np.float32))
    bblk = np.kron(eye32, b.T.astype(np.float32))
    # Band reduce pattern zsum[4g+c, g] = 1: with tile_position=(0,32q)
    # the matmul sums the 4 components of each group into partition band
    # [32q, 32q+32).
    zsum = np.kron(eye32, np.ones((4, 1), dtype=np.float32))
    return ablk, bblk, zsum


def _to_component_major(x):
    """x [BC,4] -> [128, NCOL] fp16: column N holds batch rows
    [32N, 32N+32) x 4 comps on the 128 partitions."""
    return np.ascontiguousarray(
        x.astype(np.float16).reshape(NCOL, 128).T
    )


def _pack_chunks(ablk, bblk, zsum, r_cm, m_cm):
    """Pack consts + supertile data into the per-chunk arrays."""
    def st_rm(st):
        return [r_cm[:, 512 * st : 512 * (st + 1)],
                m_cm[:, 512 * st : 512 * (st + 1)]]

    segs = {
        "s0": [ablk, zsum, bblk] + st_rm(0),
        "a0": st_rm(4),
        "g0": st_rm(2),
        "s1": st_rm(1),
        "a1": st_rm(5),
        "g1": st_rm(3),
        "s2": st_rm(6),
        "g2": st_rm(7)[:1],
        "s3": st_rm(7)[1:],
    }
    out = {}
    for name, c0, c1, _ in CHUNKS:
        arr = np.concatenate(segs[name], axis=1)
        assert arr.shape == (128, c1 - c0), (name, arr.shape)
        out[name] = np.ascontiguousarray(arr.astype(np.float16))
    return out


def _from_out32(y):
    """y [2, 128, 512] -> [BC]: value for supertile st = 4h+q, col n, group g
    lives at y[h, 32q+g, n]; batch b = 16384*st + 32n + g."""
    return np.ascontiguousarray(
        y.astype(np.float32).reshape(2, 4, 32, 512).transpose(0, 1, 3, 2)
    ).reshape(-1)


def _build_program():
    nc = bacc.Bacc("TRN2", target_bir_lowering=False, debug=False)
    chunk_d = {
        name: nc.dram_tensor(name, [128, c1 - c0], F16, kind="ExternalInput")
        for name, c0, c1, _ in CHUNKS
    }
    out_d = nc.dram_tensor("out", [2, 128, 512], F16, kind="ExternalOutput")

    out_lo_d = out_d.ap()[0]
    out_hi_d = out_d.ap()[1]

    with tile.TileContext(nc) as tc:
        with (
            tc.tile_pool(name="io", bufs=1) as iopool,
            tc.tile_pool(name="work", bufs=4) as wpool,
            tc.tile_pool(name="psum", bufs=2, space=bass.MemorySpace.PSUM) as ppool,
        ):
            dd_t = iopool.tile([128, DD_COLS], F16, name="dd_t")
            ablk = dd_t[:, COL_A : COL_A + 128]
            zsum = dd_t[:, COL_Z : COL_Z + 32]
            bblk = dd_t[:, COL_B : COL_B + 128]
            out_lo = iopool.tile([128, 512], F16, name="out_lo")
            out_hi = iopool.tile([128, 512], F16, name="out_hi")
            warm_t = iopool.tile([128, 128], F16, name="warm_t")

            # Zero the warm-up operand on DVE (exits the preamble idle).
            nc.vector.memset(warm_t[:], 0.0)

            # Input chunks across the three rings; the ordering-only
            # edges keep each ring's FIFO in consumption order.
            qeng = [nc.sync, nc.scalar, nc.gpsimd]
            prevq = [None, None, None]
            all_dmas = []
            for j, (name, c0, c1, qi) in enumerate(CHUNKS):
                dma = qeng[qi].dma_start(
                    dd_t[:, c0:c1], chunk_d[name].ap()[:, :]
                )
                if prevq[qi] is not None:
                    add_dep_helper(dma.ins, prevq[qi].ins, sync=False, reason="q")
                prevq[qi] = dma
                # Issue-depth throttle: with every chunk's descriptors
                # queued upfront, the SDMA engines interleave ALL of them
                # and every chunk's semaphore convoys to the end of the
                # stream (starving the PE mid-kernel).  Bounding the
                # outstanding chunks to 4 forces progressive completion.
                if j >= 5:
                    add_dep_helper(
                        dma.ins, all_dmas[j - 5].ins, sync=True, reason="throttle"
                    )
                all_dmas.append(dma)

            # HAM warm-up: N=128 dummy matmuls keep the PE busy through
            # the DMA ramp so the real matmuls run at 2.4 GHz.  Two
            # alternating PSUM buffers so they pipeline back-to-back.
            warm_a = ppool.tile([128, 128], F32, name="warm_a", bufs=1)
            warm_b = ppool.tile([128, 128], F32, name="warm_b", bufs=1)
            prev_mm = None
            for w in range(N_WARM):
                mm = nc.tensor.matmul(
                    (warm_a if w % 2 else warm_b)[:], warm_t[:], warm_t[:],
                    start=True, stop=True,
                )
                if prev_mm is not None:
                    add_dep_helper(mm.ins, prev_mm.ins, sync=False, reason="warm")
                prev_mm = mm

            ored = [None, None]
            seen = [0, 0]

            def emit_red(st, s_sb):
                # Band reduce, emitted one supertile late so its square's
                # semaphore is already satisfied (an isolated sem-waiting
                # matmul costs ~380 ns vs ~216 pipelined).
                q = st % 4
                h = st // 4
                if ored[h] is None:
                    ored[h] = ppool.tile([128, 512], F32, name=f"ored{h}", bufs=1)
                seen[h] += 1
                nc.tensor.matmul(
                    ored[h][32 * q : 32 * (q + 1), :], zsum, s_sb[:],
                    start=True, stop=True, tile_position=(0, 32 * q),
                )
                if seen[h] == 4 and h == 0:
                    # Full-width PSUM -> SBUF fp16 copy with the -1 fold
                    # on DVE (Scalar stays on the square chain).
                    nc.vector.tensor_scalar_add(out_lo[:], ored[0][:], -1.0)
                    odma = nc.sync.dma_start(out_lo_d, out_lo[:])
                    # Keep the store behind all Sync-ring input descgens
                    # (a sem-waiting store would stall them).
                    add_dep_helper(odma.ins, prevq[0].ins, sync=False, reason="q")
                    prevq[0] = odma

            pend = None
            for st in ST_ORDER:
                rc = _ST_RCOL[st]
                phi = ppool.tile([128, 512], F32, name="phi", bufs=4)
                mm = nc.tensor.matmul(
                    phi[:], ablk, dd_t[:, rc : rc + 512], start=True, stop=False
                )
                if prev_mm is not None:
                    # Ordering-only edge: real matmuls go behind the warm
                    # burst so a sem-waiting real LDW never stalls the PE
                    # FIFO in front of ready dummies.
                    add_dep_helper(mm.ins, prev_mm.ins, sync=False, reason="warm")
                    prev_mm = None
                nc.tensor.matmul(
                    phi[:], bblk, dd_t[:, rc + 512 : rc + 1024],
                    start=False, stop=True,
                )
                if pend is not None:
                    emit_red(*pend)

                if st == ST_ORDER[-1]:
                    # Tail supertile: split square/reduce/copy into column
                    # halves so each stage starts as soon as the first half
                    # of the chain is done, and the -1 copies run on BOTH
                    # Scalar and Vector in parallel.
                    s_sb = wpool.tile([128, 512], F16, name="s_sb")
                    q = st % 4
                    for chalf in (slice(0, 256), slice(256, 512)):
                        nc.scalar.activation(
                            s_sb[:, chalf], phi[:, chalf],
                            mybir.ActivationFunctionType.Square,
                        )
                        nc.tensor.matmul(
                            ored[1][32 * q : 32 * (q + 1), chalf],
                            zsum, s_sb[:, chalf],
                            start=True, stop=True, tile_position=(0, 32 * q),
                        )
                    nc.vector.tensor_scalar_add(
                        out_hi[:, 0:256], ored[1][:, 0:256], -1.0
                    )
                    nc.scalar.activation(
                        out_hi[:, 256:512], ored[1][:, 256:512],
                        mybir.ActivationFunctionType.Copy, bias=-1.0,
                    )
                    # Issue the final store from Scalar: its descgen sits
                    # right behind the Copy half in the same FIFO, saving
                    # a cross-engine sem-release hop (~0.4 us).
                    odma = nc.scalar.dma_start(out_hi_d, out_hi[:])
                    add_dep_helper(odma.ins, prevq[1].ins, sync=False, reason="q")
                    prevq[1] = odma
                    pend = None
                elif st == ST_ORDER[-2]:
                    # Second-to-last supertile: square on the idle DVE
                    # (PSUM copy + multiply; DVE has no Square activation
                    # and tensor_tensor cannot read PSUM twice) so the
                    # Scalar queue is free when the tail supertile's
                    # squares arrive.
                    tmp = wpool.tile([128, 512], F16, name="s_tmp", bufs=1)
                    nc.vector.tensor_copy(tmp[:], phi[:])
                    s_sb = wpool.tile([128, 512], F16, name="s_sb")
                    nc.vector.tensor_tensor(
                        s_sb[:], phi[:], tmp[:], mybir.AluOpType.mult
                    )
                    pend = (st, s_sb)
                else:
                    s_sb = wpool.tile([128, 512], F16, name="s_sb")
                    nc.scalar.activation(
                        s_sb[:], phi[:], mybir.ActivationFunctionType.Square
                    )
                    pend = (st, s_sb)
    nc.compile()
    return nc


_PROG_CACHE = None


def _get_program():
    global _PROG_CACHE
    if _PROG_CACHE is None:
        _PROG_CACHE = _build_program()
    return _PROG_CACHE


def _run(ry_params, rz_params, states_real, states_imag, **hw_kwargs):
    ablk, bblk, zsum = _host_consts(ry_params, rz_params)
    ablk = ablk.astype(np.float16)
    bblk = bblk.astype(np.float16)
    zsum = zsum.astype(np.float16)
    states_real = np.ascontiguousarray(states_real, dtype=np.float32)
    states_imag = np.ascontiguousarray(states_imag, dtype=np.float32)
    in_maps = []
    for k in range(N_CORES):
        sl = slice(k * BC, (k + 1) * BC)
        r_cm = _to_component_major(states_real[sl])
        m_cm = _to_component_major(states_imag[sl])
        in_maps.append(_pack_chunks(ablk, bblk, zsum, r_cm, m_cm))
    nc = _get_program()
    res = bass_utils.run_bass_kernel_spmd(
        nc, in_maps, core_ids=list(range(N_CORES)), **hw_kwargs
    )
    out = np.concatenate(
        [_from_out32(res.results[k]["out"]) for k in range(N_CORES)]
    ).astype(np.float32)
    return out, res


def kernel(ry_params, rz_params, states_real, states_imag):
    out, _ = _run(ry_params, rz_params, states_real, states_imag)
    return out
